# revision 9
# baseline (speedup 1.0000x reference)
"""Trainium2 Bass kernel for nn_AttentionHead (B=4, T=2048, D=1024, H=16).

Math shortcut (exact, validated vs reference):
  pooled[b] = (concat_h[ (w*r_h)^T E_h V_h ] + bv) @ Wo + bo
where E_h = exp(Q_h K_h^T / 8) (no max-subtraction needed: |scores| < ~3),
r = 1/rowsum(E), w[t] = (1/(H*T)) sum_{h,q} E_h[q,t] r_q  (head-avg column
sums of softmax), so the full attn@V [B,H,T,T]x[T,HD] and the [B*T,D]@Wo
matmuls are never materialized.

Sharding: 8 cores = (batch b = core//2) x (head-group g = core%2, 8 heads
each). w mixes all 16 heads of a batch -> one tiny [2048] f32 AllReduce
between core pairs mid-kernel. Host sums the two per-batch partial outputs
and adds the exact bias correction bv@Wo + bo.

Perf notes (cost-model driven):
 - x arrives PRE-TRANSPOSED bf16 from host (plain DMA loads, no
   DmaTranspose) and all weights arrive bf16 -> no f32 staging copies.
 - exp row-sums balanced between ScalarE accum_out (1 of 4 tiles) and DVE
   reduces so neither engine saturates; 1/rowsum is batched per half-head.
 - E is spilled in PAIRED tiles [128, 2*2048] fp8; pass-2 re-reads are ALL
   pre-issued through two ring pools (one live during the stream, one
   reusing pass-1 SBUF from the AllReduce bubble on) so they pipeline at
   DMA bandwidth instead of serializing on latency.
 - All PSUM->SBUF evictions on VectorE; score scale 1/8 folded into Wq
   host-side so ScalarE does exps only.
"""

import os
import sys

for _p in ("/opt/trn_rl_repo",):
    if _p not in sys.path and os.path.isdir(_p):
        sys.path.insert(0, _p)

from contextlib import ExitStack

import ml_dtypes
import numpy as np

import concourse.bass as bass
import concourse.mybir as mybir
import concourse.tile as tile
from concourse import bacc
from concourse.bass_utils import run_bass_kernel_spmd

FP32 = mybir.dt.float32
BF16 = mybir.dt.bfloat16
F8 = mybir.dt.float8e4
AF = mybir.ActivationFunctionType

P = 128
B, T, D, H = 4, 2048, 1024, 16
HD = D // H          # 64
NH = 8               # heads per core
NHD = NH * HD        # 512 cols per core
TQ = T // P          # 16 q-chunks
NP = TQ // 2         # 8 q-chunk pairs per head
MC = D // P          # 8 contraction chunks for projections

N_EARLY = 14         # pass-2 read pairs prefetched during the stream
N_LATE = 20          # ring depth for the remaining pass-2 reads


def _body(tc, xt_d, wq_d, wk_d, wv_d, wo_d, bqs_d, bkc_d, out_d,
          single_core=False):
    nc = tc.nc
    with ExitStack() as ctx:
        pers = ctx.enter_context(tc.tile_pool(name="pers", bufs=1))

        def ptile(shape, dtype, name):
            return pers.tile(shape, dtype, name=name, tag=name)

        QT = [ptile([P, T], BF16, f"QT{i}") for i in range(4)]
        KT = [ptile([P, T], BF16, f"KT{i}") for i in range(4)]
        Vt = [ptile([P, NHD], BF16, f"V{i}") for i in range(TQ)]
        wo_bf = [ptile([P, D], BF16, f"wo{i}") for i in range(4)]
        Za = [ptile([P, TQ], FP32, f"Za{h}") for h in range(NH)]
        Zb = [ptile([P, TQ], FP32, f"Zb{h}") for h in range(NH)]
        rV = [ptile([P, TQ], FP32, f"rV{h}") for h in range(NH)]
        rB = [ptile([P, TQ], BF16, f"rB{h}") for h in range(NH)]
        gB = [ptile([P, TQ], BF16, f"gB{h}") for h in range(NH)]
        w_col = ptile([P, TQ], FP32, "w_col")
        biasq = ptile([P, 4], FP32, "biasq")
        biask = ptile([P, 4], FP32, "biask")
        zeros_bf = ptile([P, P], BF16, "zeros_bf")
        c_sb = ptile([P, TQ], FP32, "c_sb")

        nc.gpsimd.memset(zeros_bf, 0.0)
        nc.gpsimd.dma_start(biasq, bqs_d.rearrange("(c p) -> p c", p=P))
        nc.gpsimd.dma_start(biask, bkc_d.rearrange("(c p) -> p c", p=P))

        # early prefetch pool for pass-2 E reads (paired tiles)
        E2pool = ctx.enter_context(tc.tile_pool(name="E2pool", bufs=N_EARLY))
        dram = ctx.enter_context(tc.tile_pool(name="dram", bufs=1, space="DRAM"))
        E_spill = dram.tile([NH * T, T], F8, name="E_spill", tag="E_spill")
        c_bounce = dram.tile([1, T], FP32, name="c_bounce", tag="c_bounce")
        w_bounce = dram.tile([1, T], FP32, name="w_bounce", tag="w_bounce")

        e2_pre = {}

        def prefetch_pair(pool, eng, h, pc):
            E2 = pool.tile([P, 2 * T], F8, name=f"E2_{h}_{pc}", tag="E2")
            eng.dma_start(
                E2.rearrange("p (two t) -> p two t", t=T),
                E_spill[h * T + pc * 2 * P:h * T + (pc + 1) * 2 * P, :]
                .rearrange("(two p) t -> p two t", p=P))
            e2_pre[(h, pc)] = E2

        READ_ORDER = [(h, pc) for h in range(NH) for pc in range(NP)]

        with ExitStack() as p1:
            xT = [p1.enter_context(tc.tile_pool(name=f"xTp{m}", bufs=1)).tile(
                [P, T], BF16, name=f"xT{m}", tag=f"xT{m}") for m in range(MC)]
            wq_bf = [p1.enter_context(tc.tile_pool(name=f"wqp{m}", bufs=1)).tile(
                [P, NHD], BF16, name=f"wq{m}", tag=f"wq{m}") for m in range(MC)]
            wk_bf = [p1.enter_context(tc.tile_pool(name=f"wkp{m}", bufs=1)).tile(
                [P, NHD], BF16, name=f"wk{m}", tag=f"wk{m}") for m in range(MC)]
            wv_bf = [p1.enter_context(tc.tile_pool(name=f"wvp{m}", bufs=1)).tile(
                [P, NHD], BF16, name=f"wv{m}", tag=f"wv{m}") for m in range(MC)]
            Epool = p1.enter_context(tc.tile_pool(name="Epool", bufs=8))
            psS = p1.enter_context(tc.tile_pool(name="psS", bufs=3, space="PSUM"))
            psC = p1.enter_context(tc.tile_pool(name="psC", bufs=1, space="PSUM"))
            c_ps = psC.tile([P, TQ], FP32, name="c_ps", tag="c_ps")

            # ---- direct bf16 loads, interleaved so the first projection
            # ---- sub-bursts can start as soon as their chunks land ----
            for m in range(MC):
                nc.sync.dma_start(xT[m], xt_d[m * P:(m + 1) * P, :])
                nc.sync.dma_start(wq_bf[m], wq_d[m * P:(m + 1) * P, :])
                nc.sync.dma_start(wk_bf[m], wk_d[m * P:(m + 1) * P, :])

            # ---- projections, pipelined one d-chunk ahead of attention;
            # ---- each (Q|K, qq) group split into two 4-matmul sub-bursts
            # ---- so ScalarE's 2-exp PSUM buffer never drains.
            proj_ps = {}

            def proj_sub(dc, sub):
                grp, half = sub // 2, sub % 2
                w_bf, out_t, bias_t = ((wq_bf, QT, biasq) if grp < 4 else
                                       (wk_bf, KT, biask))
                qq = grp % 4
                if half == 0:
                    proj_ps[dc] = psS.tile([P, 1024], FP32,
                                           name=f"pj{dc}_{grp}", tag="S")
                ps = proj_ps[dc]
                for m in range(4 * half, 4 * half + 4):
                    nc.tensor.matmul(ps[:, :512],
                                     lhsT=w_bf[m][:, dc * P:(dc + 1) * P],
                                     rhs=xT[m][:, qq * 512:(qq + 1) * 512],
                                     start=(m == 0), stop=(m == MC - 1))
                if half == 1:
                    nc.vector.tensor_scalar_add(
                        out_t[dc][:, qq * 512:(qq + 1) * 512],
                        ps[:, :512], bias_t[:, dc:dc + 1])

            nc.tensor.matmul(c_ps, lhsT=zeros_bf, rhs=zeros_bf[:, :TQ],
                             start=True, stop=False)

            for sub in range(16):
                proj_sub(0, sub)
            for m in range(MC):
                nc.sync.dma_start(wv_bf[m], wv_d[m * P:(m + 1) * P, :])
            # c-matmuls for a half-head run one half later (their rB is
            # produced in one batched reciprocal per half).
            pending_c = []
            c_left = [NH * TQ * TQ]

            def c_mms(h, qc, E_sb, off):
                for kc in range(TQ):
                    c_left[0] -= 1
                    nc.tensor.matmul(
                        c_ps[:, kc:kc + 1],
                        lhsT=E_sb[:, off + kc * P:off + (kc + 1) * P],
                        rhs=rB[h][:, qc:qc + 1],
                        start=False, stop=(c_left[0] == 0))

            for dc in range(4):
                for h in (2 * dc, 2 * dc + 1):
                    ro = (h % 2) * HD
                    E_pair = None
                    for qc in range(TQ):
                        # one projection sub-burst of the NEXT d-chunk per unit
                        if h == 2 * dc and dc < 3:
                            proj_sub(dc + 1, qc)
                        if qc % 2 == 0:
                            E_pair = Epool.tile([P, 2 * T], F8,
                                                name=f"E_{h}_{qc}", tag="E")
                        off = (qc % 2) * T
                        for sh in range(2):
                            ps = psS.tile([P, 1024], FP32,
                                          name=f"S_{h}_{qc}_{sh}", tag="S")
                            for kq in range(2):
                                nc.tensor.matmul(
                                    ps[:, kq * 512:(kq + 1) * 512],
                                    lhsT=QT[dc][ro:ro + HD, qc * P:(qc + 1) * P],
                                    rhs=KT[dc][ro:ro + HD,
                                               sh * 1024 + kq * 512:
                                               sh * 1024 + (kq + 1) * 512],
                                    start=True, stop=True)
                            # row-sum halves balanced across engines: one
                            # tile in four is summed free on ScalarE.
                            if sh == 0 and qc % 4 == 1:
                                nc.scalar.activation(
                                    E_pair[:, off + sh * 1024:
                                           off + (sh + 1) * 1024],
                                    ps, AF.Exp,
                                    accum_out=Za[h][:, qc:qc + 1])
                            else:
                                nc.scalar.activation(
                                    E_pair[:, off + sh * 1024:
                                           off + (sh + 1) * 1024],
                                    ps, AF.Exp)
                                tgt = Za if sh == 0 else Zb
                                nc.vector.reduce_sum(
                                    tgt[h][:, qc:qc + 1],
                                    E_pair[:, off + sh * 1024:
                                           off + (sh + 1) * 1024],
                                    axis=mybir.AxisListType.X)
                        if qc % 2 == 1:
                            nc.sync.dma_start(
                                E_spill[h * T + (qc - 1) * P:
                                        h * T + (qc + 1) * P, :]
                                .rearrange("(two p) t -> p two t", p=P),
                                E_pair.rearrange("p (two t) -> p two t", t=T))
                            # in-stream slice of pass-2 reads: emitted right
                            # after the matching spill so the dep tracker
                            # orders read-after-write; the pool ring holds
                            # them resident until pass 2.
                            if h * NP + qc // 2 < N_EARLY:
                                prefetch_pair(E2pool, nc.gpsimd, h, qc // 2)
                        # batched r for the completed half-head
                        if qc % NP == NP - 1:
                            s = qc - (NP - 1)
                            sl = slice(s, s + NP)
                            nc.vector.tensor_add(rV[h][:, sl], Za[h][:, sl],
                                                 Zb[h][:, sl])
                            nc.vector.reciprocal(rV[h][:, sl], rV[h][:, sl])
                            nc.vector.tensor_copy(rB[h][:, sl], rV[h][:, sl])
                        pending_c.append((h, qc, E_pair, off))
                        # drain one c-burst per unit, lagging one half-head
                        if len(pending_c) > NP:
                            c_mms(*pending_c.pop(0))
            while pending_c:
                c_mms(*pending_c.pop(0))

            # ---- V (fills the AllReduce bubble) ----
            for ti in range(TQ):
                ps = psS.tile([P, 1024], FP32, name=f"pv{ti}", tag="S")
                for m in range(MC):
                    nc.tensor.matmul(ps[:, :512],
                                     lhsT=xT[m][:, ti * P:(ti + 1) * P],
                                     rhs=wv_bf[m], start=(m == 0),
                                     stop=(m == MC - 1))
                nc.vector.tensor_copy(Vt[ti], ps[:, :512])

            for m in range(4):
                nc.sync.dma_start(wo_bf[m], wo_d[m * P:(m + 1) * P, :])

            nc.scalar.activation(c_sb, c_ps, AF.Copy, scale=1.0 / (H * T))
            nc.gpsimd.dma_start(
                c_bounce[:].rearrange("a (p c) -> (a p) c", p=P), c_sb)
            if single_core:
                nc.sync.dma_start(w_bounce, c_bounce)
            else:
                nc.gpsimd.collective_compute(
                    "AllReduce", mybir.AluOpType.add,
                    replica_groups=[[0, 1], [2, 3], [4, 5], [6, 7]],
                    ins=[c_bounce[:].opt()], outs=[w_bounce[:].opt()])
            nc.gpsimd.dma_start(
                w_col, w_bounce[:].rearrange("a (p c) -> (a p) c", p=P))

        # -------- pass 2: uT = E^T(w*r) via E-stationary matmuls, ------
        # -------- pooledT = V^T u via V-stationary, then @ Wo ----------
        with ExitStack() as p2:
            # ring pool reusing pass-1 SBUF: its reads wait on the last
            # pass-1 readers of that space, i.e. they start flowing at the
            # AllReduce bubble and pipeline behind consumption after that.
            E2late = p2.enter_context(tc.tile_pool(name="E2late", bufs=N_LATE))
            small = p2.enter_context(tc.tile_pool(name="small", bufs=2))
            psU = p2.enter_context(tc.tile_pool(name="psU", bufs=2, space="PSUM"))
            psP = p2.enter_context(tc.tile_pool(name="psP", bufs=1, space="PSUM"))

            for h, pc in READ_ORDER[N_EARLY:]:
                prefetch_pair(E2late, nc.sync, h, pc)

            pooledT_ps = psP.tile([P, 4], FP32, name="pooledT_ps",
                                  tag="pooledT_ps")
            nc.tensor.matmul(pooledT_ps, lhsT=zeros_bf, rhs=zeros_bf[:, :4],
                             start=True, stop=False)

            for h in range(NH):
                gf = small.tile([P, TQ], FP32, name=f"gf{h}", tag="gf")
                nc.vector.tensor_mul(gf, w_col, rV[h])
                nc.vector.tensor_copy(gB[h], gf)

            def pooled_mms(h, u_bf):
                ro, co = (h % 2) * HD, h // 2
                for kc in range(TQ):
                    nc.tensor.matmul(pooledT_ps[ro:ro + HD, co:co + 1],
                                     lhsT=Vt[kc][:, h * HD:(h + 1) * HD],
                                     rhs=u_bf[:, kc:kc + 1],
                                     start=False,
                                     stop=(kc == TQ - 1 and h == NH - 1))

            prev_u = [None]
            for h in range(NH):
                u_ps = psU.tile([P, TQ], FP32, name=f"u_ps{h}", tag="u_ps")
                nc.tensor.matmul(u_ps, lhsT=zeros_bf, rhs=zeros_bf[:, :TQ],
                                 start=True, stop=False)
                for pc in range(NP):
                    E2 = e2_pre.pop((h, pc))
                    for half in range(2):
                        qc = 2 * pc + half
                        off = half * T
                        for kc in range(TQ):
                            nc.tensor.matmul(
                                u_ps[:, kc:kc + 1],
                                lhsT=E2[:, off + kc * P:off + (kc + 1) * P],
                                rhs=gB[h][:, qc:qc + 1],
                                start=False,
                                stop=(pc == NP - 1 and half == 1
                                      and kc == TQ - 1))
                u_bf = small.tile([P, TQ], BF16, name=f"u_bf{h}", tag="u_bf",
                                  bufs=3)
                nc.vector.tensor_copy(u_bf, u_ps)
                if prev_u[0] is not None:
                    pooled_mms(h - 1, prev_u[0])
                prev_u[0] = u_bf
            pooled_mms(NH - 1, prev_u[0])

            pooledT_bf = small.tile([P, 4], BF16, name="pooledT_bf",
                                    tag="pooledT_bf")
            nc.vector.tensor_copy(pooledT_bf, pooledT_ps)

            part_ps = psU.tile([1, D], FP32, name="part_ps", tag="part_ps")
            for mc in range(4):
                for hf in range(2):
                    nc.tensor.matmul(part_ps[0:1, hf * 512:(hf + 1) * 512],
                                     lhsT=pooledT_bf[:, mc:mc + 1],
                                     rhs=wo_bf[mc][:, hf * 512:(hf + 1) * 512],
                                     start=(mc == 0), stop=(mc == 3))
            out_sb = small.tile([1, D], FP32, name="out_sb", tag="out_sb")
            nc.vector.tensor_copy(out_sb, part_ps)
            nc.sync.dma_start(out_d[:], out_sb)


_NC_CACHE = {}


def build_nc(single_core=False):
    if single_core in _NC_CACHE:
        return _NC_CACHE[single_core]
    nc = bacc.Bacc("TRN2", target_bir_lowering=False, debug=False,
                   enable_asserts=False, num_devices=1 if single_core else 8)
    xt_d = nc.dram_tensor("xt", [D, T], BF16, kind="ExternalInput")
    wq_d = nc.dram_tensor("wq", [D, NHD], BF16, kind="ExternalInput")
    wk_d = nc.dram_tensor("wk", [D, NHD], BF16, kind="ExternalInput")
    wv_d = nc.dram_tensor("wv", [D, NHD], BF16, kind="ExternalInput")
    wo_d = nc.dram_tensor("wo", [NHD, D], BF16, kind="ExternalInput")
    bqs_d = nc.dram_tensor("bqs", [NHD], FP32, kind="ExternalInput")
    bkc_d = nc.dram_tensor("bkc", [NHD], FP32, kind="ExternalInput")
    out_d = nc.dram_tensor("out", [1, D], FP32, kind="ExternalOutput")
    with tile.TileContext(nc) as tc:
        _body(tc, xt_d.ap(), wq_d.ap(), wk_d.ap(), wv_d.ap(), wo_d.ap(),
              bqs_d.ap(), bkc_d.ap(), out_d.ap(), single_core=single_core)
    nc.compile()
    _NC_CACHE[single_core] = nc
    return nc


def make_in_maps(x, Wq, bq, Wk, bk, Wv, bv, Wo, bo):
    in_maps = []
    for core in range(8):
        b, g = core // 2, core % 2
        cs = slice(g * NHD, (g + 1) * NHD)
        in_maps.append({
            "xt": np.ascontiguousarray(x[b].T).astype(ml_dtypes.bfloat16),
            "wq": np.ascontiguousarray(Wq[:, cs] * np.float32(0.125)).astype(
                ml_dtypes.bfloat16),
            "wk": np.ascontiguousarray(Wk[:, cs]).astype(ml_dtypes.bfloat16),
            "wv": np.ascontiguousarray(Wv[:, cs]).astype(ml_dtypes.bfloat16),
            "wo": np.ascontiguousarray(Wo[cs, :]).astype(ml_dtypes.bfloat16),
            "bqs": np.ascontiguousarray(bq[cs]) * np.float32(0.125),
            "bkc": np.ascontiguousarray(bk[cs]),
        })
    return in_maps


def kernel(x, Wq, bq, Wk, bk, Wv, bv, Wo, bo, _results_hook=None):
    x, Wq, bq, Wk, bk, Wv, bv, Wo, bo = (
        np.asarray(a, dtype=np.float32)
        for a in (x, Wq, bq, Wk, bk, Wv, bv, Wo, bo))
    nc = build_nc()
    in_maps = make_in_maps(x, Wq, bq, Wk, bk, Wv, bv, Wo, bo)
    res = run_bass_kernel_spmd(nc, in_maps, core_ids=list(range(8)))
    if _results_hook is not None:
        _results_hook(res)
    parts = [res.results[c]["out"][0] for c in range(8)]
    correction = bv.astype(np.float32) @ Wo.astype(np.float32) + bo
    out = np.stack([parts[2 * b] + parts[2 * b + 1] for b in range(B)])
    return (out + correction[None, :]).astype(np.float32)


# revision 12
# speedup vs baseline: 1.0174x; 1.0174x over previous
"""Trainium2 Bass kernel for nn_AttentionHead (B=4, T=2048, D=1024, H=16).

Math shortcut (exact, validated vs reference):
  pooled[b] = (concat_h[ (w*r_h)^T E_h V_h ] + bv) @ Wo + bo
where E_h = exp(Q_h K_h^T / 8) (no max-subtraction needed: |scores| < ~3),
r = 1/rowsum(E), w[t] = (1/(H*T)) sum_{h,q} E_h[q,t] r_q  (head-avg column
sums of softmax), so the full attn@V [B,H,T,T]x[T,HD] and the [B*T,D]@Wo
matmuls are never materialized.

Sharding: 8 cores = (batch b = core//2) x (head-group g = core%2, 8 heads
each). w mixes all 16 heads of a batch -> one tiny [2048] f32 AllReduce
between core pairs mid-kernel. Host sums the two per-batch partial outputs
and adds the exact bias correction bv@Wo + bo.

Perf notes (cost-model driven):
 - x arrives PRE-TRANSPOSED bf16 from host (plain DMA loads, no
   DmaTranspose) and all weights arrive bf16 -> no f32 staging copies.
 - exp row-sums balanced between ScalarE accum_out (1 of 4 tiles) and DVE
   reduces so neither engine saturates; 1/rowsum is batched per half-head.
 - E is spilled in PAIRED tiles [128, 2*2048] fp8; pass-2 re-reads are ALL
   pre-issued through two ring pools (one live during the stream, one
   reusing pass-1 SBUF from the AllReduce bubble on) so they pipeline at
   DMA bandwidth instead of serializing on latency.
 - All PSUM->SBUF evictions on VectorE; score scale 1/8 folded into Wq
   host-side so ScalarE does exps only.
"""

import os
import sys

for _p in ("/opt/trn_rl_repo",):
    if _p not in sys.path and os.path.isdir(_p):
        sys.path.insert(0, _p)

from contextlib import ExitStack

import ml_dtypes
import numpy as np

import concourse.bass as bass
import concourse.mybir as mybir
import concourse.tile as tile
from concourse import bacc
from concourse.bass_utils import run_bass_kernel_spmd

FP32 = mybir.dt.float32
BF16 = mybir.dt.bfloat16
F8 = mybir.dt.float8e4
AF = mybir.ActivationFunctionType

P = 128
B, T, D, H = 4, 2048, 1024, 16
HD = D // H          # 64
NH = 8               # heads per core
NHD = NH * HD        # 512 cols per core
TQ = T // P          # 16 q-chunks
NP = TQ // 2         # 8 q-chunk pairs per head
MC = D // P          # 8 contraction chunks for projections

N_EARLY = 14         # pass-2 read pairs prefetched during the stream
N_LATE = 20          # ring depth for the remaining pass-2 reads


def _body(tc, xt_d, wq_d, wk_d, wv_d, wo_d, bqs_d, bkc_d, out_d,
          single_core=False):
    nc = tc.nc
    with ExitStack() as ctx:
        pers = ctx.enter_context(tc.tile_pool(name="pers", bufs=1))

        def ptile(shape, dtype, name):
            return pers.tile(shape, dtype, name=name, tag=name)

        QT = [ptile([P, T], BF16, f"QT{i}") for i in range(4)]
        KT = [ptile([P, T], BF16, f"KT{i}") for i in range(4)]
        Vt = [ptile([P, NHD], BF16, f"V{i}") for i in range(TQ)]
        wo_bf = [ptile([P, D], BF16, f"wo{i}") for i in range(4)]
        Za = [ptile([P, TQ], FP32, f"Za{h}") for h in range(NH)]
        Zb = [ptile([P, TQ], FP32, f"Zb{h}") for h in range(NH)]
        rV = [ptile([P, TQ], FP32, f"rV{h}") for h in range(NH)]
        rB = [ptile([P, TQ], BF16, f"rB{h}") for h in range(NH)]
        gB = [ptile([P, TQ], BF16, f"gB{h}") for h in range(NH)]
        w_col = ptile([P, TQ], FP32, "w_col")
        biasq = ptile([P, 4], FP32, "biasq")
        biask = ptile([P, 4], FP32, "biask")
        zeros_bf = ptile([P, P], BF16, "zeros_bf")
        c_sb = ptile([P, TQ], FP32, "c_sb")

        nc.gpsimd.memset(zeros_bf, 0.0)
        nc.gpsimd.dma_start(biasq, bqs_d.rearrange("(c p) -> p c", p=P))
        nc.gpsimd.dma_start(biask, bkc_d.rearrange("(c p) -> p c", p=P))

        # early prefetch pool for pass-2 E reads (paired tiles)
        E2pool = ctx.enter_context(tc.tile_pool(name="E2pool", bufs=N_EARLY))
        dram = ctx.enter_context(tc.tile_pool(name="dram", bufs=1, space="DRAM"))
        E_spill = dram.tile([NH * T, T], F8, name="E_spill", tag="E_spill")
        c_bounce = dram.tile([1, T], FP32, name="c_bounce", tag="c_bounce")
        w_bounce = dram.tile([1, T], FP32, name="w_bounce", tag="w_bounce")

        e2_pre = {}

        def prefetch_pair(pool, eng, h, pc):
            E2 = pool.tile([P, 2 * T], F8, name=f"E2_{h}_{pc}", tag="E2")
            eng.dma_start(
                E2.rearrange("p (two t) -> p two t", t=T),
                E_spill[h * T + pc * 2 * P:h * T + (pc + 1) * 2 * P, :]
                .rearrange("(two p) t -> p two t", p=P))
            e2_pre[(h, pc)] = E2

        READ_ORDER = [(h, pc) for h in range(NH) for pc in range(NP)]

        with ExitStack() as p1:
            xT = [p1.enter_context(tc.tile_pool(name=f"xTp{m}", bufs=1)).tile(
                [P, T], BF16, name=f"xT{m}", tag=f"xT{m}") for m in range(MC)]
            wq_bf = [p1.enter_context(tc.tile_pool(name=f"wqp{m}", bufs=1)).tile(
                [P, NHD], BF16, name=f"wq{m}", tag=f"wq{m}") for m in range(MC)]
            wk_bf = [p1.enter_context(tc.tile_pool(name=f"wkp{m}", bufs=1)).tile(
                [P, NHD], BF16, name=f"wk{m}", tag=f"wk{m}") for m in range(MC)]
            wv_bf = [p1.enter_context(tc.tile_pool(name=f"wvp{m}", bufs=1)).tile(
                [P, NHD], BF16, name=f"wv{m}", tag=f"wv{m}") for m in range(MC)]
            Epool = p1.enter_context(tc.tile_pool(name="Epool", bufs=8))
            zfold = p1.enter_context(tc.tile_pool(name="zfold", bufs=3))
            psS = p1.enter_context(tc.tile_pool(name="psS", bufs=3, space="PSUM"))
            psC = p1.enter_context(tc.tile_pool(name="psC", bufs=1, space="PSUM"))
            c_ps = psC.tile([P, TQ], FP32, name="c_ps", tag="c_ps")

            # ---- direct bf16 loads, interleaved so the first projection
            # ---- sub-bursts can start as soon as their chunks land ----
            for m in range(MC):
                nc.sync.dma_start(xT[m], xt_d[m * P:(m + 1) * P, :])
                nc.sync.dma_start(wq_bf[m], wq_d[m * P:(m + 1) * P, :])
                nc.sync.dma_start(wk_bf[m], wk_d[m * P:(m + 1) * P, :])

            # ---- projections, pipelined one d-chunk ahead of attention;
            # ---- each (Q|K, qq) group split into two 4-matmul sub-bursts
            # ---- so ScalarE's 2-exp PSUM buffer never drains.
            proj_ps = {}

            def proj_sub(dc, sub):
                grp, half = sub // 2, sub % 2
                w_bf, out_t, bias_t = ((wq_bf, QT, biasq) if grp < 4 else
                                       (wk_bf, KT, biask))
                qq = grp % 4
                if half == 0:
                    proj_ps[dc] = psS.tile([P, 1024], FP32,
                                           name=f"pj{dc}_{grp}", tag="S")
                ps = proj_ps[dc]
                for m in range(4 * half, 4 * half + 4):
                    nc.tensor.matmul(ps[:, :512],
                                     lhsT=w_bf[m][:, dc * P:(dc + 1) * P],
                                     rhs=xT[m][:, qq * 512:(qq + 1) * 512],
                                     start=(m == 0), stop=(m == MC - 1))
                if half == 1:
                    nc.vector.tensor_scalar_add(
                        out_t[dc][:, qq * 512:(qq + 1) * 512],
                        ps[:, :512], bias_t[:, dc:dc + 1])

            nc.tensor.matmul(c_ps, lhsT=zeros_bf, rhs=zeros_bf[:, :TQ],
                             start=True, stop=False)

            for sub in range(16):
                proj_sub(0, sub)
            for m in range(MC):
                nc.sync.dma_start(wv_bf[m], wv_d[m * P:(m + 1) * P, :])
            # c-matmuls for a half-head run one half later (their rB is
            # produced in one batched reciprocal per half).
            pending_c = []
            c_left = [NH * TQ * TQ]

            def c_mms(h, qc, E_sb, off):
                for kc in range(TQ):
                    c_left[0] -= 1
                    nc.tensor.matmul(
                        c_ps[:, kc:kc + 1],
                        lhsT=E_sb[:, off + kc * P:off + (kc + 1) * P],
                        rhs=rB[h][:, qc:qc + 1],
                        start=False, stop=(c_left[0] == 0))

            for dc in range(4):
                for h in (2 * dc, 2 * dc + 1):
                    ro = (h % 2) * HD
                    E_pair = None
                    for qc in range(TQ):
                        # one projection sub-burst of the NEXT d-chunk per unit
                        if h == 2 * dc and dc < 3:
                            proj_sub(dc + 1, qc)
                        if qc % 2 == 0:
                            E_pair = Epool.tile([P, 2 * T], F8,
                                                name=f"E_{h}_{qc}", tag="E")
                        off = (qc % 2) * T
                        for sh in range(2):
                            ps = psS.tile([P, 1024], FP32,
                                          name=f"S_{h}_{qc}_{sh}", tag="S")
                            for kq in range(2):
                                nc.tensor.matmul(
                                    ps[:, kq * 512:(kq + 1) * 512],
                                    lhsT=QT[dc][ro:ro + HD, qc * P:(qc + 1) * P],
                                    rhs=KT[dc][ro:ro + HD,
                                               sh * 1024 + kq * 512:
                                               sh * 1024 + (kq + 1) * 512],
                                    start=True, stop=True)
                            # row-sum halves split three ways: a few summed
                            # free on ScalarE accum, most pre-folded on
                            # GPSIMD (fp8+fp8->bf16 halves) so DVE reduces
                            # 512 bf16 elems in 2x mode, rest direct on DVE.
                            base = off + sh * 1024
                            eslice = E_pair[:, base:base + 1024]
                            tgt = (Za if sh == 0 else Zb)[h][:, qc:qc + 1]
                            if sh == 0 and qc % 8 == 1:
                                nc.scalar.activation(eslice, ps, AF.Exp,
                                                     accum_out=tgt)
                            else:
                                nc.scalar.activation(eslice, ps, AF.Exp)
                                if sh == 1 or qc % 4 == 3:
                                    tmp = zfold.tile([P, 512], BF16,
                                                     name=f"zf{h}_{qc}_{sh}",
                                                     tag="zf")
                                    nc.gpsimd.tensor_add(
                                        tmp, E_pair[:, base:base + 512],
                                        E_pair[:, base + 512:base + 1024])
                                    nc.vector.reduce_sum(
                                        tgt, tmp, axis=mybir.AxisListType.X)
                                else:
                                    nc.vector.reduce_sum(
                                        tgt, eslice,
                                        axis=mybir.AxisListType.X)
                        if qc % 2 == 1:
                            nc.sync.dma_start(
                                E_spill[h * T + (qc - 1) * P:
                                        h * T + (qc + 1) * P, :]
                                .rearrange("(two p) t -> p two t", p=P),
                                E_pair.rearrange("p (two t) -> p two t", t=T))
                            # in-stream slice of pass-2 reads: emitted right
                            # after the matching spill so the dep tracker
                            # orders read-after-write; the pool ring holds
                            # them resident until pass 2.
                            if h * NP + qc // 2 < N_EARLY:
                                prefetch_pair(E2pool, nc.gpsimd, h, qc // 2)
                        # batched r for the completed half-head
                        if qc % NP == NP - 1:
                            s = qc - (NP - 1)
                            sl = slice(s, s + NP)
                            nc.vector.tensor_add(rV[h][:, sl], Za[h][:, sl],
                                                 Zb[h][:, sl])
                            nc.vector.reciprocal(rV[h][:, sl], rV[h][:, sl])
                            nc.vector.tensor_copy(rB[h][:, sl], rV[h][:, sl])
                        pending_c.append((h, qc, E_pair, off))
                        # drain one c-burst per unit, lagging one half-head
                        if len(pending_c) > NP:
                            c_mms(*pending_c.pop(0))
            while pending_c:
                c_mms(*pending_c.pop(0))

            # ---- V (fills the AllReduce bubble) ----
            for ti in range(TQ):
                ps = psS.tile([P, 1024], FP32, name=f"pv{ti}", tag="S")
                for m in range(MC):
                    nc.tensor.matmul(ps[:, :512],
                                     lhsT=xT[m][:, ti * P:(ti + 1) * P],
                                     rhs=wv_bf[m], start=(m == 0),
                                     stop=(m == MC - 1))
                nc.vector.tensor_copy(Vt[ti], ps[:, :512])

            for m in range(4):
                nc.sync.dma_start(wo_bf[m], wo_d[m * P:(m + 1) * P, :])

            nc.scalar.activation(c_sb, c_ps, AF.Copy, scale=1.0 / (H * T))
            nc.gpsimd.dma_start(
                c_bounce[:].rearrange("a (p c) -> (a p) c", p=P), c_sb)
            if single_core:
                nc.sync.dma_start(w_bounce, c_bounce)
            else:
                nc.gpsimd.collective_compute(
                    "AllReduce", mybir.AluOpType.add,
                    replica_groups=[[0, 1], [2, 3], [4, 5], [6, 7]],
                    ins=[c_bounce[:].opt()], outs=[w_bounce[:].opt()])
            nc.gpsimd.dma_start(
                w_col, w_bounce[:].rearrange("a (p c) -> (a p) c", p=P))

        # -------- pass 2: uT = E^T(w*r) via E-stationary matmuls, ------
        # -------- pooledT = V^T u via V-stationary, then @ Wo ----------
        with ExitStack() as p2:
            # ring pool reusing pass-1 SBUF: its reads wait on the last
            # pass-1 readers of that space, i.e. they start flowing at the
            # AllReduce bubble and pipeline behind consumption after that.
            E2late = p2.enter_context(tc.tile_pool(name="E2late", bufs=N_LATE))
            small = p2.enter_context(tc.tile_pool(name="small", bufs=2))
            psU = p2.enter_context(tc.tile_pool(name="psU", bufs=2, space="PSUM"))
            psP = p2.enter_context(tc.tile_pool(name="psP", bufs=1, space="PSUM"))

            for h, pc in READ_ORDER[N_EARLY:]:
                prefetch_pair(E2late, nc.sync, h, pc)

            pooledT_ps = psP.tile([P, 4], FP32, name="pooledT_ps",
                                  tag="pooledT_ps")
            nc.tensor.matmul(pooledT_ps, lhsT=zeros_bf, rhs=zeros_bf[:, :4],
                             start=True, stop=False)

            for h in range(NH):
                gf = small.tile([P, TQ], FP32, name=f"gf{h}", tag="gf")
                nc.vector.tensor_mul(gf, w_col, rV[h])
                nc.vector.tensor_copy(gB[h], gf)

            def pooled_mms(h, u_bf):
                ro, co = (h % 2) * HD, h // 2
                for kc in range(TQ):
                    nc.tensor.matmul(pooledT_ps[ro:ro + HD, co:co + 1],
                                     lhsT=Vt[kc][:, h * HD:(h + 1) * HD],
                                     rhs=u_bf[:, kc:kc + 1],
                                     start=False,
                                     stop=(kc == TQ - 1 and h == NH - 1))

            prev_u = [None]
            for h in range(NH):
                u_ps = psU.tile([P, TQ], FP32, name=f"u_ps{h}", tag="u_ps")
                nc.tensor.matmul(u_ps, lhsT=zeros_bf, rhs=zeros_bf[:, :TQ],
                                 start=True, stop=False)
                for pc in range(NP):
                    E2 = e2_pre.pop((h, pc))
                    for half in range(2):
                        qc = 2 * pc + half
                        off = half * T
                        for kc in range(TQ):
                            nc.tensor.matmul(
                                u_ps[:, kc:kc + 1],
                                lhsT=E2[:, off + kc * P:off + (kc + 1) * P],
                                rhs=gB[h][:, qc:qc + 1],
                                start=False,
                                stop=(pc == NP - 1 and half == 1
                                      and kc == TQ - 1))
                u_bf = small.tile([P, TQ], BF16, name=f"u_bf{h}", tag="u_bf",
                                  bufs=3)
                nc.vector.tensor_copy(u_bf, u_ps)
                if prev_u[0] is not None:
                    pooled_mms(h - 1, prev_u[0])
                prev_u[0] = u_bf
            pooled_mms(NH - 1, prev_u[0])

            pooledT_bf = small.tile([P, 4], BF16, name="pooledT_bf",
                                    tag="pooledT_bf")
            nc.vector.tensor_copy(pooledT_bf, pooledT_ps)

            part_ps = psU.tile([1, D], FP32, name="part_ps", tag="part_ps")
            for mc in range(4):
                for hf in range(2):
                    nc.tensor.matmul(part_ps[0:1, hf * 512:(hf + 1) * 512],
                                     lhsT=pooledT_bf[:, mc:mc + 1],
                                     rhs=wo_bf[mc][:, hf * 512:(hf + 1) * 512],
                                     start=(mc == 0), stop=(mc == 3))
            out_sb = small.tile([1, D], FP32, name="out_sb", tag="out_sb")
            nc.vector.tensor_copy(out_sb, part_ps)
            nc.sync.dma_start(out_d[:], out_sb)


_NC_CACHE = {}


def build_nc(single_core=False):
    if single_core in _NC_CACHE:
        return _NC_CACHE[single_core]
    nc = bacc.Bacc("TRN2", target_bir_lowering=False, debug=False,
                   enable_asserts=False, num_devices=1 if single_core else 8)
    xt_d = nc.dram_tensor("xt", [D, T], BF16, kind="ExternalInput")
    wq_d = nc.dram_tensor("wq", [D, NHD], BF16, kind="ExternalInput")
    wk_d = nc.dram_tensor("wk", [D, NHD], BF16, kind="ExternalInput")
    wv_d = nc.dram_tensor("wv", [D, NHD], BF16, kind="ExternalInput")
    wo_d = nc.dram_tensor("wo", [NHD, D], BF16, kind="ExternalInput")
    bqs_d = nc.dram_tensor("bqs", [NHD], FP32, kind="ExternalInput")
    bkc_d = nc.dram_tensor("bkc", [NHD], FP32, kind="ExternalInput")
    out_d = nc.dram_tensor("out", [1, D], FP32, kind="ExternalOutput")
    with tile.TileContext(nc) as tc:
        _body(tc, xt_d.ap(), wq_d.ap(), wk_d.ap(), wv_d.ap(), wo_d.ap(),
              bqs_d.ap(), bkc_d.ap(), out_d.ap(), single_core=single_core)
    nc.compile()
    _NC_CACHE[single_core] = nc
    return nc


def make_in_maps(x, Wq, bq, Wk, bk, Wv, bv, Wo, bo):
    in_maps = []
    for core in range(8):
        b, g = core // 2, core % 2
        cs = slice(g * NHD, (g + 1) * NHD)
        in_maps.append({
            "xt": np.ascontiguousarray(x[b].T).astype(ml_dtypes.bfloat16),
            "wq": np.ascontiguousarray(Wq[:, cs] * np.float32(0.125)).astype(
                ml_dtypes.bfloat16),
            "wk": np.ascontiguousarray(Wk[:, cs]).astype(ml_dtypes.bfloat16),
            "wv": np.ascontiguousarray(Wv[:, cs]).astype(ml_dtypes.bfloat16),
            "wo": np.ascontiguousarray(Wo[cs, :]).astype(ml_dtypes.bfloat16),
            "bqs": np.ascontiguousarray(bq[cs]) * np.float32(0.125),
            "bkc": np.ascontiguousarray(bk[cs]),
        })
    return in_maps


def kernel(x, Wq, bq, Wk, bk, Wv, bv, Wo, bo, _results_hook=None):
    x, Wq, bq, Wk, bk, Wv, bv, Wo, bo = (
        np.asarray(a, dtype=np.float32)
        for a in (x, Wq, bq, Wk, bk, Wv, bv, Wo, bo))
    nc = build_nc()
    in_maps = make_in_maps(x, Wq, bq, Wk, bk, Wv, bv, Wo, bo)
    res = run_bass_kernel_spmd(nc, in_maps, core_ids=list(range(8)))
    if _results_hook is not None:
        _results_hook(res)
    parts = [res.results[c]["out"][0] for c in range(8)]
    correction = bv.astype(np.float32) @ Wo.astype(np.float32) + bo
    out = np.stack([parts[2 * b] + parts[2 * b + 1] for b in range(B)])
    return (out + correction[None, :]).astype(np.float32)


# revision 15
# speedup vs baseline: 1.0426x; 1.0248x over previous
"""Trainium2 Bass kernel for nn_AttentionHead (B=4, T=2048, D=1024, H=16).

Math shortcut (exact, validated vs reference):
  pooled[b] = (concat_h[ (w*r_h)^T E_h V_h ] + bv) @ Wo + bo
where E_h = exp(Q_h K_h^T / 8) (no max-subtraction needed: |scores| < ~3),
r = 1/rowsum(E), w[t] = (1/(H*T)) sum_{h,q} E_h[q,t] r_q  (head-avg column
sums of softmax), so the full attn@V [B,H,T,T]x[T,HD] and the [B*T,D]@Wo
matmuls are never materialized.

Sharding: 8 cores = (batch b = core//2) x (head-group g = core%2, 8 heads
each). w mixes all 16 heads of a batch -> one tiny [2048] f32 AllReduce
between core pairs mid-kernel. Host sums the two per-batch partial outputs
and adds the exact bias correction bv@Wo + bo.

Perf notes (cost-model driven):
 - x arrives PRE-TRANSPOSED bf16 from host (plain DMA loads, no
   DmaTranspose) and all weights arrive bf16 -> no f32 staging copies.
 - exp row-sums balanced between ScalarE accum_out (1 of 4 tiles) and DVE
   reduces so neither engine saturates; 1/rowsum is batched per half-head.
 - E is spilled in PAIRED tiles [128, 2*2048] fp8; pass-2 re-reads are ALL
   pre-issued through two ring pools (one live during the stream, one
   reusing pass-1 SBUF from the AllReduce bubble on) so they pipeline at
   DMA bandwidth instead of serializing on latency.
 - All PSUM->SBUF evictions on VectorE; score scale 1/8 folded into Wq
   host-side so ScalarE does exps only.
"""

import os
import sys

for _p in ("/opt/trn_rl_repo",):
    if _p not in sys.path and os.path.isdir(_p):
        sys.path.insert(0, _p)

from contextlib import ExitStack

import ml_dtypes
import numpy as np

import concourse.bass as bass
import concourse.mybir as mybir
import concourse.tile as tile
from concourse import bacc
from concourse.bass_utils import run_bass_kernel_spmd

FP32 = mybir.dt.float32
BF16 = mybir.dt.bfloat16
F8 = mybir.dt.float8e4
AF = mybir.ActivationFunctionType

P = 128
B, T, D, H = 4, 2048, 1024, 16
HD = D // H          # 64
NH = 8               # heads per core
NHD = NH * HD        # 512 cols per core
TQ = T // P          # 16 q-chunks
NP = TQ // 2         # 8 q-chunk pairs per head
MC = D // P          # 8 contraction chunks for projections

N_EARLY = 14         # pass-2 read pairs prefetched during the stream
N_LATE = 13          # ring depth for the remaining pass-2 reads


def _body(tc, xt_d, wq_d, wk_d, wv_d, wo_d, bqs_d, bkc_d, out_d,
          single_core=False):
    nc = tc.nc
    with ExitStack() as ctx:
        pers = ctx.enter_context(tc.tile_pool(name="pers", bufs=1))

        def ptile(shape, dtype, name):
            return pers.tile(shape, dtype, name=name, tag=name)

        QT = [ptile([P, T], BF16, f"QT{i}") for i in range(4)]
        KT = [ptile([P, T], BF16, f"KT{i}") for i in range(4)]
        Vt = [ptile([P, NHD], BF16, f"V{i}") for i in range(TQ)]
        wo_bf = [ptile([P, D], BF16, f"wo{i}") for i in range(4)]
        Za = [ptile([P, TQ], FP32, f"Za{h}") for h in range(NH)]
        Zb = [ptile([P, TQ], FP32, f"Zb{h}") for h in range(NH)]
        rV = [ptile([P, TQ], FP32, f"rV{h}") for h in range(NH)]
        rB = [ptile([P, TQ], BF16, f"rB{h}") for h in range(NH)]
        gB = [ptile([P, TQ], BF16, f"gB{h}") for h in range(NH)]
        w_col = ptile([P, TQ], FP32, "w_col")
        biasq = ptile([P, 4], FP32, "biasq")
        biask = ptile([P, 4], FP32, "biask")
        zeros_bf = ptile([P, P], BF16, "zeros_bf")
        c_sb = ptile([P, TQ], FP32, "c_sb")

        nc.gpsimd.memset(zeros_bf, 0.0)
        nc.gpsimd.dma_start(biasq, bqs_d.rearrange("(c p) -> p c", p=P))
        nc.gpsimd.dma_start(biask, bkc_d.rearrange("(c p) -> p c", p=P))

        # early prefetch pool for pass-2 E reads (paired tiles)
        E2pool = ctx.enter_context(tc.tile_pool(name="E2pool", bufs=N_EARLY))
        dram = ctx.enter_context(tc.tile_pool(name="dram", bufs=1, space="DRAM"))
        E_spill = dram.tile([NH * T, T], F8, name="E_spill", tag="E_spill")
        c_bounce = dram.tile([1, T], FP32, name="c_bounce", tag="c_bounce")
        w_bounce = dram.tile([1, T], FP32, name="w_bounce", tag="w_bounce")

        e2_pre = {}

        def prefetch_pair(pool, eng, h, pc):
            E2 = pool.tile([P, 2 * T], F8, name=f"E2_{h}_{pc}", tag="E2")
            eng.dma_start(
                E2.rearrange("p (two t) -> p two t", t=T),
                E_spill[h * T + pc * 2 * P:h * T + (pc + 1) * 2 * P, :]
                .rearrange("(two p) t -> p two t", p=P))
            e2_pre[(h, pc)] = E2

        # head 7 is recomputed from QT/KT in pass 2 (its exps fill the
        # AllReduce bubble on the otherwise-idle ScalarE), so it is never
        # spilled nor re-read.
        REC_H = NH - 1
        READ_ORDER = [(h, pc) for h in range(REC_H) for pc in range(NP)]

        with ExitStack() as p1:
            xT = [p1.enter_context(tc.tile_pool(name=f"xTp{m}", bufs=1)).tile(
                [P, T], BF16, name=f"xT{m}", tag=f"xT{m}") for m in range(MC)]
            wq_bf = [p1.enter_context(tc.tile_pool(name=f"wqp{m}", bufs=1)).tile(
                [P, NHD], BF16, name=f"wq{m}", tag=f"wq{m}") for m in range(MC)]
            wk_bf = [p1.enter_context(tc.tile_pool(name=f"wkp{m}", bufs=1)).tile(
                [P, NHD], BF16, name=f"wk{m}", tag=f"wk{m}") for m in range(MC)]
            wv_bf = [p1.enter_context(tc.tile_pool(name=f"wvp{m}", bufs=1)).tile(
                [P, NHD], BF16, name=f"wv{m}", tag=f"wv{m}") for m in range(MC)]
            Epool = p1.enter_context(tc.tile_pool(name="Epool", bufs=8))
            zfold = p1.enter_context(tc.tile_pool(name="zfold", bufs=3))
            psS = p1.enter_context(tc.tile_pool(name="psS", bufs=3, space="PSUM"))
            psC = p1.enter_context(tc.tile_pool(name="psC", bufs=1, space="PSUM"))
            c_ps = psC.tile([P, TQ], FP32, name="c_ps", tag="c_ps")

            # ---- direct bf16 loads, interleaved so the first projection
            # ---- sub-bursts can start as soon as their chunks land ----
            for m in range(MC):
                nc.sync.dma_start(xT[m], xt_d[m * P:(m + 1) * P, :])
                nc.sync.dma_start(wq_bf[m], wq_d[m * P:(m + 1) * P, :])
                nc.sync.dma_start(wk_bf[m], wk_d[m * P:(m + 1) * P, :])

            # ---- projections, pipelined one d-chunk ahead of attention;
            # ---- each (Q|K, qq) group split into two 4-matmul sub-bursts
            # ---- so ScalarE's 2-exp PSUM buffer never drains.
            proj_ps = {}

            def proj_sub(dc, sub):
                grp, half = sub // 2, sub % 2
                w_bf, out_t, bias_t = ((wq_bf, QT, biasq) if grp < 4 else
                                       (wk_bf, KT, biask))
                qq = grp % 4
                if half == 0:
                    proj_ps[dc] = psS.tile([P, 1024], FP32,
                                           name=f"pj{dc}_{grp}", tag="S")
                ps = proj_ps[dc]
                for m in range(4 * half, 4 * half + 4):
                    nc.tensor.matmul(ps[:, :512],
                                     lhsT=w_bf[m][:, dc * P:(dc + 1) * P],
                                     rhs=xT[m][:, qq * 512:(qq + 1) * 512],
                                     start=(m == 0), stop=(m == MC - 1))
                if half == 1:
                    nc.vector.tensor_scalar_add(
                        out_t[dc][:, qq * 512:(qq + 1) * 512],
                        ps[:, :512], bias_t[:, dc:dc + 1])

            nc.tensor.matmul(c_ps, lhsT=zeros_bf, rhs=zeros_bf[:, :TQ],
                             start=True, stop=False)

            for sub in range(16):
                proj_sub(0, sub)
            for m in range(MC):
                nc.sync.dma_start(wv_bf[m], wv_d[m * P:(m + 1) * P, :])
            # c-matmuls for a half-head run one half later (their rB is
            # produced in one batched reciprocal per half).
            pending_c = []
            c_left = [NH * TQ * TQ]

            def c_mms(h, qc, E_sb, off):
                for kc in range(TQ):
                    c_left[0] -= 1
                    nc.tensor.matmul(
                        c_ps[:, kc:kc + 1],
                        lhsT=E_sb[:, off + kc * P:off + (kc + 1) * P],
                        rhs=rB[h][:, qc:qc + 1],
                        start=False, stop=(c_left[0] == 0))

            for dc in range(4):
                for h in (2 * dc, 2 * dc + 1):
                    ro = (h % 2) * HD
                    E_pair = None
                    for qc in range(TQ):
                        # one projection sub-burst of the NEXT d-chunk per unit
                        if h == 2 * dc and dc < 3:
                            proj_sub(dc + 1, qc)
                        if qc % 2 == 0:
                            E_pair = Epool.tile([P, 2 * T], F8,
                                                name=f"E_{h}_{qc}", tag="E")
                        off = (qc % 2) * T
                        for sh in range(2):
                            ps = psS.tile([P, 1024], FP32,
                                          name=f"S_{h}_{qc}_{sh}", tag="S")
                            for kq in range(2):
                                nc.tensor.matmul(
                                    ps[:, kq * 512:(kq + 1) * 512],
                                    lhsT=QT[dc][ro:ro + HD, qc * P:(qc + 1) * P],
                                    rhs=KT[dc][ro:ro + HD,
                                               sh * 1024 + kq * 512:
                                               sh * 1024 + (kq + 1) * 512],
                                    start=True, stop=True)
                            # row-sum halves split three ways: a few summed
                            # free on ScalarE accum, most pre-folded on
                            # GPSIMD (fp8+fp8->bf16 halves) so DVE reduces
                            # 512 bf16 elems in 2x mode, rest direct on DVE.
                            base = off + sh * 1024
                            eslice = E_pair[:, base:base + 1024]
                            tgt = (Za if sh == 0 else Zb)[h][:, qc:qc + 1]
                            if sh == 0 and qc % 8 == 1:
                                nc.scalar.activation(eslice, ps, AF.Exp,
                                                     accum_out=tgt)
                            else:
                                nc.scalar.activation(eslice, ps, AF.Exp)
                                if sh == 1 or qc % 4 == 3:
                                    tmp = zfold.tile([P, 512], BF16,
                                                     name=f"zf{h}_{qc}_{sh}",
                                                     tag="zf")
                                    nc.gpsimd.tensor_add(
                                        tmp, E_pair[:, base:base + 512],
                                        E_pair[:, base + 512:base + 1024])
                                    nc.vector.reduce_sum(
                                        tgt, tmp, axis=mybir.AxisListType.X)
                                else:
                                    nc.vector.reduce_sum(
                                        tgt, eslice,
                                        axis=mybir.AxisListType.X)
                        if qc % 2 == 1 and h != REC_H:
                            nc.sync.dma_start(
                                E_spill[h * T + (qc - 1) * P:
                                        h * T + (qc + 1) * P, :]
                                .rearrange("(two p) t -> p two t", p=P),
                                E_pair.rearrange("p (two t) -> p two t", t=T))
                            # in-stream slice of pass-2 reads: emitted right
                            # after the matching spill so the dep tracker
                            # orders read-after-write; the pool ring holds
                            # them resident until pass 2.
                            if h * NP + qc // 2 < N_EARLY:
                                prefetch_pair(E2pool, nc.gpsimd, h, qc // 2)
                        # batched r for the completed half-head
                        if qc % NP == NP - 1:
                            s = qc - (NP - 1)
                            sl = slice(s, s + NP)
                            nc.vector.tensor_add(rV[h][:, sl], Za[h][:, sl],
                                                 Zb[h][:, sl])
                            nc.vector.reciprocal(rV[h][:, sl], rV[h][:, sl])
                            nc.vector.tensor_copy(rB[h][:, sl], rV[h][:, sl])
                        pending_c.append((h, qc, E_pair, off))
                        # drain one c-burst per unit, lagging one half-head
                        if len(pending_c) > NP:
                            c_mms(*pending_c.pop(0))
            while pending_c:
                c_mms(*pending_c.pop(0))

            # ---- V (fills the AllReduce bubble) ----
            for ti in range(TQ):
                ps = psS.tile([P, 1024], FP32, name=f"pv{ti}", tag="S")
                for m in range(MC):
                    nc.tensor.matmul(ps[:, :512],
                                     lhsT=xT[m][:, ti * P:(ti + 1) * P],
                                     rhs=wv_bf[m], start=(m == 0),
                                     stop=(m == MC - 1))
                nc.vector.tensor_copy(Vt[ti], ps[:, :512])

            for m in range(4):
                nc.sync.dma_start(wo_bf[m], wo_d[m * P:(m + 1) * P, :])

            nc.scalar.activation(c_sb, c_ps, AF.Copy, scale=1.0 / (H * T))
            nc.gpsimd.dma_start(
                c_bounce[:].rearrange("a (p c) -> (a p) c", p=P), c_sb)
            if single_core:
                nc.sync.dma_start(w_bounce, c_bounce)
            else:
                nc.gpsimd.collective_compute(
                    "AllReduce", mybir.AluOpType.add,
                    replica_groups=[[0, 1], [2, 3], [4, 5], [6, 7]],
                    ins=[c_bounce[:].opt()], outs=[w_bounce[:].opt()])
            nc.gpsimd.dma_start(
                w_col, w_bounce[:].rearrange("a (p c) -> (a p) c", p=P))

        # -------- pass 2: uT = E^T(w*r) via E-stationary matmuls, ------
        # -------- pooledT = V^T u via V-stationary, then @ Wo ----------
        with ExitStack() as p2:
            # ring pool reusing pass-1 SBUF: its reads wait on the last
            # pass-1 readers of that space, i.e. they start flowing at the
            # AllReduce bubble and pipeline behind consumption after that.
            E2late = p2.enter_context(tc.tile_pool(name="E2late", bufs=N_LATE))
            E7pool = p2.enter_context(tc.tile_pool(name="E7pool", bufs=NP))
            psS2 = p2.enter_context(tc.tile_pool(name="psS2", bufs=2,
                                                 space="PSUM"))
            small = p2.enter_context(tc.tile_pool(name="small", bufs=2))
            psU = p2.enter_context(tc.tile_pool(name="psU", bufs=1, space="PSUM"))
            psP = p2.enter_context(tc.tile_pool(name="psP", bufs=1, space="PSUM"))
            psF = p2.enter_context(tc.tile_pool(name="psF", bufs=1, space="PSUM"))

            # recompute head 7's E (emitted FIRST so its score matmuls and
            # exps are not stuck behind w-gated u-matmuls in the in-order
            # PE/ACT queues; they run during the AllReduce bubble)
            ro7 = (REC_H % 2) * HD
            for pc in range(NP):
                E7 = E7pool.tile([P, 2 * T], F8, name=f"E7_{pc}", tag="E7")
                for half in range(2):
                    qc = 2 * pc + half
                    off = half * T
                    for sh in range(2):
                        ps = psS2.tile([P, 1024], FP32,
                                       name=f"S7_{qc}_{sh}", tag="S7")
                        for kq in range(2):
                            nc.tensor.matmul(
                                ps[:, kq * 512:(kq + 1) * 512],
                                lhsT=QT[3][ro7:ro7 + HD, qc * P:(qc + 1) * P],
                                rhs=KT[3][ro7:ro7 + HD,
                                          sh * 1024 + kq * 512:
                                          sh * 1024 + (kq + 1) * 512],
                                start=True, stop=True)
                        nc.scalar.activation(
                            E7[:, off + sh * 1024:off + (sh + 1) * 1024],
                            ps, AF.Exp)
                e2_pre[(REC_H, pc)] = E7

            for h, pc in READ_ORDER[N_EARLY:]:
                prefetch_pair(E2late, nc.sync, h, pc)

            pooledT_ps = psP.tile([P, 4], FP32, name="pooledT_ps",
                                  tag="pooledT_ps")
            nc.tensor.matmul(pooledT_ps, lhsT=zeros_bf, rhs=zeros_bf[:, :4],
                             start=True, stop=False)

            for h in range(NH):
                gf = small.tile([P, TQ], FP32, name=f"gf{h}", tag="gf")
                nc.vector.tensor_mul(gf, w_col, rV[h])
                nc.vector.tensor_copy(gB[h], gf)

            def pooled_mms(h, u_bf):
                ro, co = (h % 2) * HD, h // 2
                for kc in range(TQ):
                    nc.tensor.matmul(pooledT_ps[ro:ro + HD, co:co + 1],
                                     lhsT=Vt[kc][:, h * HD:(h + 1) * HD],
                                     rhs=u_bf[:, kc:kc + 1],
                                     start=False,
                                     stop=(kc == TQ - 1 and h == NH - 1))

            prev_u = [None]
            for h in range(NH):
                u_ps = psU.tile([P, TQ], FP32, name=f"u_ps{h}", tag="u_ps")
                nc.tensor.matmul(u_ps, lhsT=zeros_bf, rhs=zeros_bf[:, :TQ],
                                 start=True, stop=False)
                for pc in range(NP):
                    E2 = e2_pre.pop((h, pc))
                    for half in range(2):
                        qc = 2 * pc + half
                        off = half * T
                        for kc in range(TQ):
                            nc.tensor.matmul(
                                u_ps[:, kc:kc + 1],
                                lhsT=E2[:, off + kc * P:off + (kc + 1) * P],
                                rhs=gB[h][:, qc:qc + 1],
                                start=False,
                                stop=(pc == NP - 1 and half == 1
                                      and kc == TQ - 1))
                u_bf = small.tile([P, TQ], BF16, name=f"u_bf{h}", tag="u_bf",
                                  bufs=3)
                nc.vector.tensor_copy(u_bf, u_ps)
                if prev_u[0] is not None:
                    pooled_mms(h - 1, prev_u[0])
                prev_u[0] = u_bf
            pooled_mms(NH - 1, prev_u[0])

            pooledT_bf = small.tile([P, 4], BF16, name="pooledT_bf",
                                    tag="pooledT_bf")
            nc.vector.tensor_copy(pooledT_bf, pooledT_ps)

            part_ps = psF.tile([1, D], FP32, name="part_ps", tag="part_ps")
            for mc in range(4):
                for hf in range(2):
                    nc.tensor.matmul(part_ps[0:1, hf * 512:(hf + 1) * 512],
                                     lhsT=pooledT_bf[:, mc:mc + 1],
                                     rhs=wo_bf[mc][:, hf * 512:(hf + 1) * 512],
                                     start=(mc == 0), stop=(mc == 3))
            out_sb = small.tile([1, D], FP32, name="out_sb", tag="out_sb")
            nc.vector.tensor_copy(out_sb, part_ps)
            nc.sync.dma_start(out_d[:], out_sb)


_NC_CACHE = {}


def build_nc(single_core=False):
    if single_core in _NC_CACHE:
        return _NC_CACHE[single_core]
    nc = bacc.Bacc("TRN2", target_bir_lowering=False, debug=False,
                   enable_asserts=False, num_devices=1 if single_core else 8)
    xt_d = nc.dram_tensor("xt", [D, T], BF16, kind="ExternalInput")
    wq_d = nc.dram_tensor("wq", [D, NHD], BF16, kind="ExternalInput")
    wk_d = nc.dram_tensor("wk", [D, NHD], BF16, kind="ExternalInput")
    wv_d = nc.dram_tensor("wv", [D, NHD], BF16, kind="ExternalInput")
    wo_d = nc.dram_tensor("wo", [NHD, D], BF16, kind="ExternalInput")
    bqs_d = nc.dram_tensor("bqs", [NHD], FP32, kind="ExternalInput")
    bkc_d = nc.dram_tensor("bkc", [NHD], FP32, kind="ExternalInput")
    out_d = nc.dram_tensor("out", [1, D], FP32, kind="ExternalOutput")
    with tile.TileContext(nc) as tc:
        _body(tc, xt_d.ap(), wq_d.ap(), wk_d.ap(), wv_d.ap(), wo_d.ap(),
              bqs_d.ap(), bkc_d.ap(), out_d.ap(), single_core=single_core)
    nc.compile()
    _NC_CACHE[single_core] = nc
    return nc


def make_in_maps(x, Wq, bq, Wk, bk, Wv, bv, Wo, bo):
    in_maps = []
    for core in range(8):
        b, g = core // 2, core % 2
        cs = slice(g * NHD, (g + 1) * NHD)
        in_maps.append({
            "xt": np.ascontiguousarray(x[b].T).astype(ml_dtypes.bfloat16),
            "wq": np.ascontiguousarray(Wq[:, cs] * np.float32(0.125)).astype(
                ml_dtypes.bfloat16),
            "wk": np.ascontiguousarray(Wk[:, cs]).astype(ml_dtypes.bfloat16),
            "wv": np.ascontiguousarray(Wv[:, cs]).astype(ml_dtypes.bfloat16),
            "wo": np.ascontiguousarray(Wo[cs, :]).astype(ml_dtypes.bfloat16),
            "bqs": np.ascontiguousarray(bq[cs]) * np.float32(0.125),
            "bkc": np.ascontiguousarray(bk[cs]),
        })
    return in_maps


def kernel(x, Wq, bq, Wk, bk, Wv, bv, Wo, bo, _results_hook=None):
    x, Wq, bq, Wk, bk, Wv, bv, Wo, bo = (
        np.asarray(a, dtype=np.float32)
        for a in (x, Wq, bq, Wk, bk, Wv, bv, Wo, bo))
    nc = build_nc()
    in_maps = make_in_maps(x, Wq, bq, Wk, bk, Wv, bv, Wo, bo)
    res = run_bass_kernel_spmd(nc, in_maps, core_ids=list(range(8)))
    if _results_hook is not None:
        _results_hook(res)
    parts = [res.results[c]["out"][0] for c in range(8)]
    correction = bv.astype(np.float32) @ Wo.astype(np.float32) + bo
    out = np.stack([parts[2 * b] + parts[2 * b + 1] for b in range(B)])
    return (out + correction[None, :]).astype(np.float32)
